# revision 1
# baseline (speedup 1.0000x reference)
"""Trainium2 Bass kernel for nn_MaskGen: per-sample 1x1 conv (channel dot)
+ global BatchNorm2d(1) (training-mode batch stats) + LeakyReLU(0.1).

Sharding: pure data parallel over batch B=32 -> 4 batches per core on 8 cores.
Global batch-norm stats via a tiny [128,2] AllReduce inside the kernel.

Per core:
  - feats shard viewed as [256, 25600] (row b*64+c), split into 2 "groups"
    of 2 batches (128 rows = 2 batches x 64 channels on partitions).
  - Matmul with feats as the STATIONARY side: lhsT = feats chunk [128, 128hw],
    rhs = block-diagonal sf [128, 2] (sf for the 2 batches of the group on
    disjoint 64-row halves).  out = [128 hw-partitions, 2 batches] at PSUM
    base partition 0 -> the group's mask accumulates as [128, 400]
    (col 2*ch + r, partition = hw % 128), a single PSUM bank.
  - Stats: per-partition sum + sumsq via ACT accum_out (single producer
    engine), groups combined on DVE, AllReduce of [128, 2] across 8 cores,
    then a ones-matmul reduces over partitions AND broadcasts the totals to
    all 128 partitions.
  - Normalize: y = mask*scale + shift (DVE tensor_scalar from PSUM),
    LeakyReLU as max(y, 0.1*y), then PE-transpose two [128, 100] blocks per
    (group, batch) so one DMA per output row writes contiguous 512B lines.

Sync-capacity constraints (walrus codegen): DMA instructions carry at most
ONE semaphore wait, matmul/engine instructions two.  The DMA plan keeps
every DMA at <=1 wait: feats tiles are never reused (no WAR), all
producer-dependent DMAs are first on their hardware DGE queue.
"""

import os
from contextlib import ExitStack

import numpy as np

import concourse.bass as bass
import concourse.tile as tile
from concourse import mybir
from concourse.bass_utils import run_bass_kernel_spmd

N_CORES = 8
B, C, H, W = 32, 64, 160, 160
HW = H * W                # 25600
BPC = B // N_CORES        # 4 batches per core
NG = BPC // 2             # 2 groups (pairs of batches) per core
ROWS = BPC * C            # 256 feats rows per core
N_TOT = B * HW            # 819200 elements in the batchnorm stats
P = 128                   # hw elements per matmul chunk (PE stationary cols)
NCHUNK = HW // P          # 200 chunks per group
TILE_W = 2560             # feats DMA tile width
NLOAD = HW // TILE_W      # 10 loads per group
MM_PER_LOAD = TILE_W // P  # 20 matmuls per loaded tile
TBLK = NCHUNK // 2        # 100 chunks per transpose block
EPS = 1e-5
SLOPE = 0.1

F32 = mybir.dt.float32

# compute dtype for the channel-dot matmul; bfloat16 halves HBM traffic.
# Set KERNEL_DTYPE=f32r to fall back to fp32 inputs (float32r matmul).
_DT_ENV = os.environ.get("KERNEL_DTYPE", "bf16")
IN_DT = mybir.dt.bfloat16 if _DT_ENV == "bf16" else mybir.dt.float32r
IN_DT_NP = np.dtype(mybir.dt.np(mybir.dt.bfloat16)) if _DT_ENV == "bf16" else np.dtype(np.float32)


def _body(ctx: ExitStack, tc: "tile.TileContext", feats, sf, bnwb, out, iters=1):
    nc = tc.nc
    AF = mybir.ActivationFunctionType
    ALU = mybir.AluOpType

    singles = ctx.enter_context(tc.tile_pool(name="singles", bufs=1))
    # one slot per feats tile: no slot reuse -> feats DMAs carry no WAR wait
    ftp = ctx.enter_context(tc.tile_pool(name="ftp", bufs=NG * NLOAD))
    psum = ctx.enter_context(tc.tile_pool(name="psum", bufs=1, space="PSUM"))
    work = ctx.enter_context(tc.tile_pool(name="work", bufs=4))
    norm = ctx.enter_context(tc.tile_pool(name="norm", bufs=2))
    dram = ctx.enter_context(tc.tile_pool(name="dram", bufs=1, space="DRAM"))

    # --- block-diagonal sf weights (host-precomputed): col 2g+r holds
    #     sf[2g+r,:] in rows 64r:64r+64, zeros elsewhere.  SWDGE queue 0.
    w_sb = singles.tile([128, 2 * NG], IN_DT)
    nc.gpsimd.dma_start(out=w_sb, in_=sf)

    # ones for the partition-reduce + broadcast matmul
    ones_sb = singles.tile([128, 128], F32)
    nc.vector.memset(ones_sb, 1.0)
    # PE warm-up dummies: absorb the w_sb-DMA and ones-memset waits into
    # PE's vector clock so no later matmul needs a second wait slot
    # (walrus gives the LoadWeights sub-instruction a single wait).
    warm_ps = psum.tile([128, 1], F32, tag="warm")
    nc.tensor.matmul(out=warm_ps[: 2 * NG, :], lhsT=w_sb, rhs=w_sb[:, 0:1],
                     start=True, stop=True)
    nc.tensor.matmul(out=warm_ps, lhsT=ones_sb, rhs=ones_sb[:, 0:1],
                     start=True, stop=True)

    loop_n = int(os.environ.get("KERNEL_HWLOOP", "0"))
    if loop_n > 1:
        with tc.For_i(0, loop_n, 1):
            _iter_body(nc, tc, feats, sf, bnwb, out,
                       singles=singles, ftp=ftp, psum=psum,
                       work=work, dram=dram, norm=norm,
                       w_sb=w_sb, ones_sb=ones_sb)
    else:
        for _it in range(iters):
            _iter_body(nc, tc, feats, sf, bnwb, out,
                       singles=singles, ftp=ftp, psum=psum,
                       work=work, dram=dram, norm=norm,
                       w_sb=w_sb, ones_sb=ones_sb)


def _iter_body(nc, tc, feats, sf, bnwb, out, *, singles, ftp, psum,
               work, dram, norm, w_sb, ones_sb):
    AF = mybir.ActivationFunctionType
    ALU = mybir.AluOpType
    # per-partition partials: cols [sum_g0, sumsq_g0, sum_g1, sumsq_g1]
    # written ONLY by ACT (accum_out) so consumers wait on a single engine.
    partials = singles.tile([128, 2 * NG], F32, tag="partials")

    mask_ps = []
    y0s = []
    for g in range(NG):
        mp = psum.tile([128, 2 * NCHUNK], F32, tag=f"mask{g}")
        mask_ps.append(mp)
        for l in range(NLOAD):
            ft = ftp.tile([128, TILE_W], IN_DT, tag="ft")
            nc.gpsimd.dma_start(
                out=ft,
                in_=feats[128 * g : 128 * (g + 1), TILE_W * l : TILE_W * (l + 1)],
            )
            for m in range(MM_PER_LOAD):
                ch = MM_PER_LOAD * l + m
                nc.tensor.matmul(
                    out=mp[:, 2 * ch : 2 * ch + 2],
                    lhsT=ft[:, P * m : P * (m + 1)],
                    rhs=w_sb[:, 2 * g : 2 * g + 2],
                    start=True,
                    stop=True,
                )
        # group stats on ACT only (single engine reads the PSUM mask):
        # sumsq via Square-accum, sum via Copy-accum; the Copy output is the
        # SBUF mask used by the normalize stage.
        sq = work.tile([128, 2 * NCHUNK], F32, tag="sq")
        nc.scalar.activation(
            out=sq,
            in_=mp,
            func=AF.Square,
            accum_out=partials[:, 2 * g + 1 : 2 * g + 2],
        )
        cp = work.tile([128, 2 * NCHUNK], F32, tag="cp")
        nc.scalar.activation(
            out=cp,
            in_=mp,
            func=AF.Copy,
            accum_out=partials[:, 2 * g : 2 * g + 1],
        )
        y0s.append(cp)

    # combine groups per partition: [sum, sumsq] on each partition
    pp2 = singles.tile([128, 2], F32, tag="pp2")
    nc.vector.tensor_add(out=pp2, in0=partials[:, 0:2], in1=partials[:, 2:4])

    # --- AllReduce per-partition [sum, sumsq] across the 8 cores.
    # HWDGE queue plan (8 queues, nothing wraps): cc_in q0, cc_back q1,
    # wbb q2, out-DMAs q3-q6.
    cc_in = dram.tile([128, 2], F32, tag="cc_in")
    cc_out = dram.tile([128, 2], F32, tag="cc_out")
    nc.sync.dma_start(out=cc_in[:], in_=pp2)
    nc.gpsimd.collective_compute(
        "AllReduce",
        mybir.AluOpType.add,
        replica_groups=[list(range(N_CORES))],
        ins=[cc_in.opt()],
        outs=[cc_out.opt()],
    )
    allred = singles.tile([128, 2], F32, tag="allred")
    nc.sync.dma_start(out=allred, in_=cc_out[:])

    # partition-reduce AND broadcast: stats_ps[m, j] = sum_p allred[p, j]
    stats_ps = psum.tile([128, 2], F32, tag="stats")
    nc.tensor.matmul(
        out=stats_ps,
        lhsT=ones_sb,
        rhs=allred,
        start=True,
        stop=True,
    )
    # single-engine (DVE) scalar-math chain: every op below has at most one
    # distinct semaphore dependency (walrus allows one wait per instruction).
    stats_sb = singles.tile([128, 2], F32, tag="stats_sb")
    nc.vector.tensor_copy(out=stats_sb, in_=stats_ps)

    # bn weight+bias broadcast to all partitions, DVE-touched so consumers
    # depend on DVE only: [128, 2] = [w, b]
    wbb_raw = singles.tile([128, 2], F32, tag="wbb_raw")
    nc.sync.dma_start(out=wbb_raw, in_=bnwb.to_broadcast([128, 2]))
    wbb = singles.tile([128, 2], F32, tag="wbb")
    nc.vector.tensor_copy(out=wbb, in_=wbb_raw)

    # --- scalar math, replicated across partitions ([128,1] tiles)
    mean = singles.tile([128, 1], F32, tag="mean")
    nc.vector.tensor_scalar_mul(out=mean, in0=stats_sb[:, 0:1], scalar1=1.0 / N_TOT)
    ex2 = singles.tile([128, 1], F32, tag="ex2")
    nc.vector.tensor_scalar_mul(out=ex2, in0=stats_sb[:, 1:2], scalar1=1.0 / N_TOT)
    msq = singles.tile([128, 1], F32, tag="msq")
    nc.vector.tensor_mul(out=msq, in0=mean, in1=mean)
    var = singles.tile([128, 1], F32, tag="var")
    nc.vector.tensor_sub(out=var, in0=ex2, in1=msq)
    eps_sb = singles.tile([128, 1], F32, tag="eps_sb")
    nc.vector.memset(eps_sb, EPS)
    std = singles.tile([128, 1], F32, tag="std")
    nc.scalar.activation(out=std, in_=var, func=AF.Sqrt, bias=eps_sb)
    inv = singles.tile([128, 1], F32, tag="inv")
    nc.vector.reciprocal(out=inv, in_=std)
    scl = singles.tile([128, 1], F32, tag="scl")
    nc.vector.tensor_mul(out=scl, in0=inv, in1=wbb[:, 0:1])
    msc = singles.tile([128, 1], F32, tag="msc")
    nc.vector.tensor_mul(out=msc, in0=mean, in1=scl)
    shf = singles.tile([128, 1], F32, tag="shf")
    nc.vector.tensor_sub(out=shf, in0=wbb[:, 1:2], in1=msc)

    # --- normalize + LeakyReLU + store (permuted layout, host un-permutes)
    # mask layout: mp[p, 2*ch + r] = mask[2g+r, 128*ch + p]
    for g in range(NG):
        y0 = y0s[g]
        y = norm.tile([128, 2 * NCHUNK], F32, tag="y")
        nc.vector.tensor_scalar(
            out=y,
            in0=y0,
            scalar1=scl,
            scalar2=shf,
            op0=ALU.mult,
            op1=ALU.add,
        )
        # LeakyReLU fused: o = max(y * SLOPE, y)
        o = norm.tile([128, 2 * NCHUNK], F32, tag="o")
        nc.vector.scalar_tensor_tensor(
            out=o, in0=y, scalar=SLOPE, in1=y, op0=ALU.mult, op1=ALU.max
        )
        # out[p, 400g + 2ch + r] = leaky(norm(mask[2g+r, 128ch+p]));
        # contiguous 1600B per-partition lines, host applies the inverse
        # permutation during unshard.
        nc.sync.dma_start(
            out=out[:, 2 * NCHUNK * g : 2 * NCHUNK * (g + 1)],
            in_=o,
        )


def _split_multi_waits(nc):
    """walrus codegen accepts one semaphore wait per instruction (each ISA
    struct embeds a single EVENTS slot).  Tile's scheduler attaches several;
    hoist all but the last onto standalone EventSemaphore instructions on the
    same engine, immediately before the original instruction."""
    n = 0
    for fn in nc.m.functions:
        for bb in fn.blocks:
            insts = list(bb.instructions)
            if not any(
                i.sync_info is not None and len(i.sync_info.on_wait) > 1
                for i in insts
            ):
                continue
            new_insts = []
            for inst in insts:
                si = inst.sync_info
                if si is not None and len(si.on_wait) > 1:
                    waits = list(si.on_wait)
                    for w in waits[:-1]:
                        n += 1
                        ev = mybir.InstEventSemaphore(
                            name=f"{inst.name}-sw{n}",
                            ins=[],
                            outs=[],
                            sync_info=mybir.SyncInfo(on_wait=[w], on_update=[]),
                        )
                        ev.engine = inst.engine
                        nc.register_instruction(ev, overwrite=True)
                        new_insts.append(ev)
                    si.on_wait = [waits[-1]]
                new_insts.append(inst)
            bb.instructions = new_insts
    return n


def build_nc(iters=None):
    if iters is None:
        iters = int(os.environ.get("KERNEL_ITERS", "1"))
    nc = bass.Bass(num_devices=N_CORES)
    feats = nc.declare_dram_parameter("feats", [ROWS, HW], IN_DT, isOutput=False)
    sf = nc.declare_dram_parameter("sf", [128, 2 * NG], IN_DT, isOutput=False)
    bnwb = nc.declare_dram_parameter("bn_wb", [1, 2], F32, isOutput=False)
    out = nc.declare_dram_parameter("out", [128, 2 * NG * NCHUNK], F32, isOutput=True)
    with tile.TileContext(nc, num_cores=N_CORES) as tc:
        with ExitStack() as ctx:
            _body(ctx, tc, feats[:], sf[:], bnwb[:], out[:], iters=iters)
    _split_multi_waits(nc)
    return nc


def make_in_maps(sf, feats, bn_weight, bn_bias):
    sf = np.asarray(sf)
    feats = np.asarray(feats)
    bnwb = np.array(
        [[np.float32(np.asarray(bn_weight).reshape(-1)[0]),
          np.float32(np.asarray(bn_bias).reshape(-1)[0])]],
        dtype=np.float32,
    )
    sf2 = np.ascontiguousarray(sf.reshape(B, C)).astype(IN_DT_NP)
    in_maps = []
    for k in range(N_CORES):
        fshard = np.ascontiguousarray(
            feats[BPC * k : BPC * (k + 1)].reshape(ROWS, HW)
        ).astype(IN_DT_NP)
        wmat = np.zeros((128, 2 * NG), dtype=IN_DT_NP)
        for g in range(NG):
            for r in range(2):
                wmat[64 * r : 64 * r + 64, 2 * g + r] = sf2[BPC * k + 2 * g + r]
        in_maps.append(
            {
                "feats": fshard,
                "sf": wmat,
                "bn_wb": bnwb,
            }
        )
    return in_maps


_NC_CACHE = {}


def get_nc():
    if "nc" not in _NC_CACHE:
        _NC_CACHE["nc"] = build_nc()
    return _NC_CACHE["nc"]


def assemble(results):
    parts = []
    for r in results:
        a = np.asarray(r["out"], dtype=np.float32).reshape(128, NG, NCHUNK, 2)
        # [p, g, ch, r] -> [g, r, ch, p] -> [BPC, HW]
        parts.append(np.ascontiguousarray(a.transpose(1, 3, 2, 0)).reshape(BPC, HW))
    return np.concatenate(parts, axis=0).reshape(B, 1, H, W).astype(np.float32)


def kernel(sf, feats, bn_weight, bn_bias):
    nc = get_nc()
    in_maps = make_in_maps(sf, feats, bn_weight, bn_bias)
    res = run_bass_kernel_spmd(nc, in_maps, list(range(N_CORES)))
    return assemble(res.results)

